# revision 44
# baseline (speedup 1.0000x reference)
"""Bahdanau-attention kernel for one TRN2 chip (8 NeuronCores, SPMD).

Math (per batch row b, sequence position s):
    att[b, s] = v . tanh(h_part[b] + enc[s, b, :] @ W_e)
    out[b, :] = softmax(att[b, :])        with h_part = hidden @ W_h + b_attn

Sharding: pure data-parallel over batch (B=32 -> 4 per core), no collectives.

Key design points:
- Host-side layout prep: the big matmul contracts over H, which must live on
  SBUF partitions, so encoder_outputs is pre-transposed to H-major on the host
  and every device DMA is one contiguous block.
- The energy matmul runs as fp8(e4m3) DoubleRow (2 weights/cell, effective
  K=256 per pass, half the matmul count of bf16).  W_e is pre-scaled by 64 on
  the host so its small values stay in fp8's normal range; the tanh activation
  rescales by 1/64 for free.  h_part / v-dot stay bf16; accumulation is fp32.
- tanh(h_part + e_part) runs on the scalar engine with the per-(q,b) bias
  folded in; [128,1024] tiles halve the per-op overhead.  Softmax skips the
  max-subtraction (|logit| <= ||v||_1 ~ 18, safe in fp32 exp).
- Software-pipelined emission: e-matmuls of block i+1 precede the
  tanh-dependent v-dot matmuls of block i-1 in the PE stream (2-block skew),
  exp is deferred one block so it never head-of-line-blocks tanh in the ACT
  FIFO, and dummy matmuls pre-warm the PE clock (HAM) during the first DMAs.
Measured: ~78 us on-chip (neuron-profile exec_time), rel err ~1.3e-2 vs the
fp32 reference (L2); max abs err ~6e-5 on a softmax output of scale ~0.1.
"""

import sys

sys.path.insert(0, "/opt/trn_rl_repo")

import numpy as np

from concourse import bacc, bass, mybir, tile
from concourse.bass_utils import run_bass_kernel_spmd

H = 512
DH = 4 * H            # 2048 (hidden feature dim)
B, S = 32, 2048
NCORES = 8
BC = B // NCORES      # 4 batch rows per core
KH = H // 128         # 4 contraction tiles over H
KD = DH // 128        # 16 contraction tiles over DH
NQ = H // 128         # 4 output quadrants of H
SBLK = 1024           # sequence positions per block
NBLK = S // SBLK      # 2 blocks per batch row
HB = 512              # half-block: psum-bank / matmul-N granularity
NCH = S // HB         # 4 per-row chunks for the softmax
F32 = mybir.dt.float32
F32R = mybir.dt.float32r
BF16 = mybir.dt.bfloat16
F8 = mybir.dt.float8e4
WE_SCALE = 64.0

_NC_CACHE = None


def _build():
    nc = bacc.Bacc(
        "TRN2", target_bir_lowering=False, debug=False, num_devices=NCORES
    )
    enc_d = nc.dram_tensor(
        "enc_t", [BC, NBLK, 128, KH, SBLK], F8, kind="ExternalInput"
    )
    hid_d = nc.dram_tensor("hid_t", [128, KD, BC], BF16, kind="ExternalInput")
    wh_d = nc.dram_tensor("w_h", [128, KD, H], BF16, kind="ExternalInput")
    we_d = nc.dram_tensor("w_e", [128, KH, H], F8, kind="ExternalInput")
    ba_d = nc.dram_tensor("b_attn", [128, NQ], F32, kind="ExternalInput")
    v_d = nc.dram_tensor("v", [128, NQ], BF16, kind="ExternalInput")
    vm_d = nc.dram_tensor("vmat", [128, NQ, NCH * BC, NCH * BC], BF16, kind="ExternalInput")
    id_d = nc.dram_tensor("ident", [BC, BC], F32, kind="ExternalInput")
    bd_d = nc.dram_tensor("bdiag", [NCH * BC, NCH * BC], F32, kind="ExternalInput")
    out_d = nc.dram_tensor("out", [BC, NCH, HB], F32, kind="ExternalOutput")

    TANH = mybir.ActivationFunctionType.Tanh
    EXP = mybir.ActivationFunctionType.Exp
    COPY = mybir.ActivationFunctionType.Copy

    with tile.TileContext(nc) as tc:
        with (
            tc.tile_pool(name="const", bufs=1) as constp,
            tc.tile_pool(name="enc", bufs=6) as encp,
            tc.tile_pool(name="energy", bufs=8) as enp,
            tc.tile_pool(name="small", bufs=1) as smallp,
            tc.tile_pool(name="psum_e", bufs=3, space=bass.MemorySpace.PSUM) as pse,
            tc.tile_pool(name="psum_s", bufs=1, space=bass.MemorySpace.PSUM) as pss,
        ):
            wh_sb = constp.tile([128, KD, H], BF16)
            nc.scalar.dma_start(wh_sb[:, 0 : KD // 2, :], wh_d[:, 0 : KD // 2, :])
            we_sb = constp.tile([128, KH, H], F8)
            for k in range(KH):
                nc.scalar.dma_start(we_sb[:, k, :], we_d[:, k, :])
            ba_sb = constp.tile([128, NQ], F32)
            nc.scalar.dma_start(ba_sb[:], ba_d[:])
            v_sb = constp.tile([128, NQ], BF16)
            nc.scalar.dma_start(v_sb[:], v_d[:])
            vm_sb = constp.tile([128, NQ, NCH * BC, NCH * BC], BF16)
            nc.scalar.dma_start(vm_sb[:], vm_d[:])
            id_sb = constp.tile([BC, BC], F32)
            nc.scalar.dma_start(id_sb[:], id_d[:])
            bd_sb = constp.tile([NCH * BC, NCH * BC], F32)
            nc.scalar.dma_start(bd_sb[:], bd_d[:])

            hptb = constp.tile([128, NQ, BC], F32)
            NR = NCH * BC            # 16 logit rows: partition 4b+c
            ex16 = smallp.tile([NR, HB], F32)
            csum = smallp.tile([NR, 1], F32)
            rs16 = smallp.tile([NR, 1], F32)
            out16 = smallp.tile([NR, HB], F32)

            ps_small = pss.tile([128, HB], F32)

            # HAM pre-warm: ~3.5 us of dummy matmuls on zeroed scratch while
            # the first DMAs are still in flight, so real matmuls start at
            # full clock (K=8/8)
            warm = constp.tile([128, 512], BF16)
            nc.vector.memset(warm[:], 0.0)
            for _ in range(18):
                nc.tensor.matmul(
                    ps_small[:, :], warm[:, 0:128], warm[:], start=True, stop=True
                )

            blocks = [(b, s) for b in range(BC) for s in range(NBLK)]
            NBLOCKS = len(blocks)
            ets = {}
            epss = {}

            def load_block(i):
                b, sblk = blocks[i]
                et = encp.tile([128, KH, SBLK], F8)
                nc.sync.dma_start(et[:], enc_d[b, sblk])
                ets[i] = et

            def emit_emm(i, qs=None):
                b, sblk = blocks[i]
                if qs is None or qs[0] == 0:
                    epss[i] = []
                et = ets[i]
                eps4 = epss[i]
                qlist = list(qs) if qs is not None else list(range(NQ))
                tiles = {}
                for q in qlist:
                    tiles[q] = pse.tile([128, SBLK], F32, name="eps", tag="eps")
                for qpair in [qlist[i : i + 2] for i in range(0, len(qlist), 2)]:
                    # stationary (j, q) constant across both halves: walrus can
                    # reuse the loaded weights for the second matmul
                    for j in range(KH // 2):
                        for q in qpair:
                            for half in range(SBLK // HB):
                                hsl = slice(half * HB, (half + 1) * HB)
                                nc.tensor.matmul(
                                    tiles[q][:, hsl],
                                    we_sb[
                                        :, 2 * j : 2 * j + 2, q * 128 : (q + 1) * 128
                                    ],
                                    et[:, 2 * j : 2 * j + 2, hsl],
                                    start=(j == 0),
                                    stop=(j == KH // 2 - 1),
                                    perf_mode=mybir.MatmulPerfMode.DoubleRow,
                                )
                for q in qlist:
                    eps4.append(tiles[q])
                if qs is None or qs[-1] == NQ - 1:
                    ets.pop(i)

            ens = {}

            def emit_tanh(i):
                b, sblk = blocks[i]
                en4 = []
                for q in range(NQ):
                    eps = epss[i][q]
                    en = enp.tile([128, SBLK], BF16)
                    nc.scalar.activation(
                        en[:],
                        eps[:],
                        TANH,
                        bias=hptb[:, q, b : b + 1],
                        scale=1.0 / WE_SCALE,
                    )
                    en4.append(en)
                ens[i] = en4
                del epss[i]

            def emit_v(i):
                # chunk c -> row c of ps_small[0:16] (bank 0).  Stationary is
                # [128,16] with v in column c and zeros elsewhere, so rows
                # other than c accumulate zero; chunk 0 q0 start=True clears
                # the bank once, everything after accumulates via has_written.
                NR = NCH * BC
                for half in range(SBLK // HB):
                    c = i * (SBLK // HB) + half
                    att_ps = ps_small[0:NR, 0:HB]
                    for q in range(NQ):
                        nc.tensor.matmul(
                            att_ps,
                            vm_sb[:, q, c, :],
                            ens[i][q][:, half * HB : (half + 1) * HB],
                            start=(c == 0 and q == 0),
                            stop=(q == NQ - 1),
                            skip_group_check=True,
                        )
                del ens[i]

            # prologue: sync queue carries only enc tiles (fp8, 256 KB each);
            # h_part matmuls interleave with block 0's e-matmuls so the tanh
            # bias is ready as early as possible
            load_block(0)
            hid_sb = constp.tile([128, KD, BC], BF16)
            nc.sync.dma_start(hid_sb[:], hid_d[:])
            nc.sync.dma_start(wh_sb[:, KD // 2 :, :], wh_d[:, KD // 2 :, :])
            load_block(1)
            hp_ps = ps_small[0:BC, 0:H]

            def emit_hp(ks):
                for k in ks:
                    nc.tensor.matmul(
                        hp_ps,
                        hid_sb[:, k, :],
                        wh_sb[:, k, :],
                        start=(k == 0),
                        stop=(k == KD - 1),
                    )

            emit_hp(range(KD))
            hp_sb = smallp.tile([BC, H], F32)
            nc.vector.tensor_copy(hp_sb[:], hp_ps)

            # transpose to [128, q, b] via PE, fold in b_attn -> tanh bias
            for q in range(NQ):
                hpt_ps = ps_small[:, q * BC : (q + 1) * BC]
                nc.tensor.transpose(
                    hpt_ps, hp_sb[:, q * 128 : (q + 1) * 128], id_sb[:]
                )
                nc.vector.tensor_scalar_add(
                    hptb[:, q, :], hpt_ps, ba_sb[:, q : q + 1]
                )
            emit_emm(0)

            # steady state, one-block skew: e-matmuls of block i+1 sit ahead of
            # block i's tanh-dependent v-dots in the PE stream
            for i in range(NBLOCKS - 1):
                if i + 2 < NBLOCKS:
                    load_block(i + 2)
                if i + 1 < NBLOCKS:
                    emit_emm(i + 1)
                emit_tanh(i)
                if i >= 1:
                    emit_v(i - 1)
            emit_v(NBLOCKS - 2)
            # final block: tanh of quadrant q immediately feeds its 2 v-dot
            # matmuls, so the drain is one tanh + pipelining instead of four
            last = NBLOCKS - 1
            NR = NCH * BC
            lb, _ = blocks[last]
            for q in range(NQ):
                en = enp.tile([128, SBLK], BF16, name="en_tail")
                nc.scalar.activation(
                    en[:],
                    epss[last][q][:],
                    TANH,
                    bias=hptb[:, q, lb : lb + 1],
                    scale=1.0 / WE_SCALE,
                )
                for half in range(SBLK // HB):
                    c = last * (SBLK // HB) + half
                    nc.tensor.matmul(
                        ps_small[0:NR, 0:HB],
                        vm_sb[:, q, c, :],
                        en[:, half * HB : (half + 1) * HB],
                        start=False,
                        stop=(q == NQ - 1),
                        skip_group_check=True,
                    )
            del epss[last]

            # one-shot softmax over all 16 logit rows [16, 512] in psum
            nc.scalar.activation(
                ex16[:], ps_small[0:NR, 0:HB], EXP, accum_out=csum[:]
            )
            rs_ps = pse.tile([128, SBLK], F32, name="eps", tag="eps")
            nc.tensor.matmul(
                rs_ps[0:NR, 0:1], bd_sb[:], csum[:], start=True, stop=True
            )
            nc.vector.reciprocal(rs16[:], rs_ps[0:NR, 0:1])
            nc.vector.tensor_scalar_mul(out16[:], ex16[:], rs16[:])
            for b in range(BC):
                nc.sync.dma_start(out_d[b], out16[NCH * b : NCH * b + NCH, :])

    nc.compile()
    return nc


def _get_nc():
    global _NC_CACHE
    if _NC_CACHE is None:
        _NC_CACHE = _build()
    return _NC_CACHE


def _prep_inputs(hidden, encoder_outputs, W_attn, b_attn, v):
    f = np.float32
    W_h = np.asarray(W_attn[:DH], dtype=f)
    W_e = np.asarray(W_attn[DH:], dtype=f)
    import ml_dtypes
    bf = ml_dtypes.bfloat16
    f8 = ml_dtypes.float8_e4m3
    wh_prep = np.ascontiguousarray(W_h.reshape(KD, 128, H).transpose(1, 0, 2)).astype(bf)
    we_prep = np.clip(
        np.ascontiguousarray(W_e.reshape(KH, 128, H).transpose(1, 0, 2)) * 64.0,
        -240.0, 240.0,
    ).astype(f8)
    ba_prep = np.ascontiguousarray(np.asarray(b_attn, dtype=f).reshape(NQ, 128).T)
    v_prep = np.ascontiguousarray(np.asarray(v, dtype=f).reshape(NQ, 128).T).astype(bf)
    NR = NCH * BC
    vmat = np.zeros((128, NQ, NR, NR), dtype=f)
    for c in range(NR):
        vmat[:, :, c, c] = np.asarray(v, dtype=f).reshape(NQ, 128).T
    vmat_prep = vmat.astype(bf)
    ident = np.eye(BC, dtype=f)
    NR = NCH * BC
    bd = np.zeros((NR, NR), dtype=f)
    for g in range(BC):
        bd[NCH * g : NCH * g + NCH, NCH * g : NCH * g + NCH] = 1.0
    hidden = np.asarray(hidden, dtype=f)
    encoder_outputs = np.asarray(encoder_outputs, dtype=f)

    in_maps = []
    for c in range(NCORES):
        b0 = c * BC
        hc = hidden[b0 : b0 + BC]                       # [BC, DH]
        hid_prep = np.ascontiguousarray(
            hc.T.reshape(KD, 128, BC).transpose(1, 0, 2)
        ).astype(bf)
        ec = encoder_outputs[:, b0 : b0 + BC, :]        # [S, BC, H]
        # enc_prep[b, sblk, p, k, si] = ec[sblk*SBLK+si, b, k*128+p]
        enc_prep = np.clip(
            np.ascontiguousarray(
                ec.transpose(1, 0, 2)
                .reshape(BC, NBLK, SBLK, KH, 128)
                .transpose(0, 1, 4, 3, 2)
            ),
            -240.0, 240.0,
        ).astype(ml_dtypes.float8_e4m3)
        in_maps.append(
            {
                "enc_t": enc_prep,
                "hid_t": hid_prep,
                "w_h": wh_prep,
                "w_e": we_prep,
                "b_attn": ba_prep,
                "v": v_prep,
                "ident": ident,
                "bdiag": bd,
                "vmat": vmat_prep,
            }
        )
    return in_maps


def _run(inputs, trace=False, **kw):
    nc = _get_nc()
    in_maps = _prep_inputs(
        inputs["hidden"],
        inputs["encoder_outputs"],
        inputs["W_attn"],
        inputs["b_attn"],
        inputs["v"],
    )
    res = run_bass_kernel_spmd(
        nc, in_maps, core_ids=list(range(NCORES)), trace=trace, **kw
    )
    out = np.concatenate(
        [r["out"].reshape(BC, S) for r in res.results], axis=0
    ).astype(np.float32)
    return out, res


def kernel(**inputs):
    out, _ = _run(inputs, trace=False)
    return out



# revision 46
# speedup vs baseline: 1.0235x; 1.0235x over previous
"""Bahdanau-attention kernel for one TRN2 chip (8 NeuronCores, SPMD).

Math (per batch row b, sequence position s):
    att[b, s] = v . tanh(h_part[b] + enc[s, b, :] @ W_e)
    out[b, :] = softmax(att[b, :])        with h_part = hidden @ W_h + b_attn

Sharding: pure data-parallel over batch (B=32 -> 4 per core), no collectives.

Key design points:
- Host-side layout prep: the big matmul contracts over H, which must live on
  SBUF partitions, so encoder_outputs is pre-transposed to H-major on the host
  and every device DMA is one contiguous block.
- The energy matmul runs as fp8(e4m3) DoubleRow (2 weights/cell, effective
  K=256 per pass, half the matmul count of bf16).  W_e is pre-scaled by 64 on
  the host so its small values stay in fp8's normal range; the tanh activation
  rescales by 1/64 for free.  h_part / v-dot stay bf16; accumulation is fp32.
- tanh(h_part + e_part) runs on the scalar engine with the per-(q,b) bias
  folded in; [128,1024] tiles halve the per-op overhead.  Softmax skips the
  max-subtraction (|logit| <= ||v||_1 ~ 18, safe in fp32 exp).
- Software-pipelined emission: e-matmuls of block i+1 precede the
  tanh-dependent v-dot matmuls of block i-1 in the PE stream (2-block skew),
  exp is deferred one block so it never head-of-line-blocks tanh in the ACT
  FIFO, and dummy matmuls pre-warm the PE clock (HAM) during the first DMAs.
Measured: ~78 us on-chip (neuron-profile exec_time), rel err ~1.3e-2 vs the
fp32 reference (L2); max abs err ~6e-5 on a softmax output of scale ~0.1.
"""

import sys

sys.path.insert(0, "/opt/trn_rl_repo")

import numpy as np

from concourse import bacc, bass, mybir, tile
from concourse.bass_utils import run_bass_kernel_spmd

H = 512
DH = 4 * H            # 2048 (hidden feature dim)
B, S = 32, 2048
NCORES = 8
BC = B // NCORES      # 4 batch rows per core
KH = H // 128         # 4 contraction tiles over H
KD = DH // 128        # 16 contraction tiles over DH
NQ = H // 128         # 4 output quadrants of H
SBLK = 1024           # sequence positions per block
NBLK = S // SBLK      # 2 blocks per batch row
HB = 512              # half-block: psum-bank / matmul-N granularity
NCH = S // HB         # 4 per-row chunks for the softmax
F32 = mybir.dt.float32
F32R = mybir.dt.float32r
BF16 = mybir.dt.bfloat16
F8 = mybir.dt.float8e4
WE_SCALE = 64.0

_NC_CACHE = None


def _build():
    nc = bacc.Bacc(
        "TRN2", target_bir_lowering=False, debug=False, num_devices=NCORES
    )
    enc_d = nc.dram_tensor(
        "enc_t", [BC, NBLK, 128, KH, SBLK], F8, kind="ExternalInput"
    )
    hid_d = nc.dram_tensor("hid_t", [128, KD, BC], BF16, kind="ExternalInput")
    wh_d = nc.dram_tensor("w_h", [128, KD, H], BF16, kind="ExternalInput")
    we_d = nc.dram_tensor("w_e", [128, KH // 2, NQ, 256], F8, kind="ExternalInput")
    ba_d = nc.dram_tensor("b_attn", [128, NQ], F32, kind="ExternalInput")
    v_d = nc.dram_tensor("v", [128, NQ], BF16, kind="ExternalInput")
    vm_d = nc.dram_tensor("vmat", [128, NQ, NCH * BC, NCH * BC], BF16, kind="ExternalInput")
    id_d = nc.dram_tensor("ident", [BC, BC], F32, kind="ExternalInput")
    bd_d = nc.dram_tensor("bdiag", [NCH * BC, NCH * BC], F32, kind="ExternalInput")
    out_d = nc.dram_tensor("out", [BC, NCH, HB], F32, kind="ExternalOutput")

    TANH = mybir.ActivationFunctionType.Tanh
    EXP = mybir.ActivationFunctionType.Exp
    COPY = mybir.ActivationFunctionType.Copy

    with tile.TileContext(nc) as tc:
        with (
            tc.tile_pool(name="const", bufs=1) as constp,
            tc.tile_pool(name="enc", bufs=6) as encp,
            tc.tile_pool(name="energy", bufs=8) as enp,
            tc.tile_pool(name="small", bufs=1) as smallp,
            tc.tile_pool(name="psum_e", bufs=3, space=bass.MemorySpace.PSUM) as pse,
            tc.tile_pool(name="psum_s", bufs=1, space=bass.MemorySpace.PSUM) as pss,
        ):
            wh_sb = constp.tile([128, KD, H], BF16)
            nc.scalar.dma_start(wh_sb[:, 0 : KD // 2, :], wh_d[:, 0 : KD // 2, :])
            we_sb = constp.tile([128, KH // 2, NQ, 256], F8)
            nc.scalar.dma_start(we_sb[:], we_d[:])
            ba_sb = constp.tile([128, NQ], F32)
            nc.scalar.dma_start(ba_sb[:], ba_d[:])
            v_sb = constp.tile([128, NQ], BF16)
            nc.scalar.dma_start(v_sb[:], v_d[:])
            vm_sb = constp.tile([128, NQ, NCH * BC, NCH * BC], BF16)
            nc.scalar.dma_start(vm_sb[:], vm_d[:])
            id_sb = constp.tile([BC, BC], F32)
            nc.scalar.dma_start(id_sb[:], id_d[:])
            bd_sb = constp.tile([NCH * BC, NCH * BC], F32)
            nc.scalar.dma_start(bd_sb[:], bd_d[:])

            hptb = constp.tile([128, NQ, BC], F32)
            NR = NCH * BC            # 16 logit rows: partition 4b+c
            ex16 = smallp.tile([NR, HB], F32)
            csum = smallp.tile([NR, 1], F32)
            rs16 = smallp.tile([NR, 1], F32)
            out16 = smallp.tile([NR, HB], F32)

            ps_small = pss.tile([128, HB], F32)

            # HAM pre-warm: ~3.5 us of dummy matmuls on zeroed scratch while
            # the first DMAs are still in flight, so real matmuls start at
            # full clock (K=8/8)
            warm = constp.tile([128, 512], BF16)
            nc.vector.memset(warm[:], 0.0)
            for _ in range(18):
                nc.tensor.matmul(
                    ps_small[:, :], warm[:, 0:128], warm[:], start=True, stop=True
                )

            blocks = [(b, s) for b in range(BC) for s in range(NBLK)]
            NBLOCKS = len(blocks)
            ets = {}
            epss = {}

            def load_block(i):
                b, sblk = blocks[i]
                et = encp.tile([128, KH, SBLK], F8)
                nc.sync.dma_start(et[:], enc_d[b, sblk])
                ets[i] = et

            def emit_emm(i, qs=None):
                b, sblk = blocks[i]
                if qs is None or qs[0] == 0:
                    epss[i] = []
                et = ets[i]
                eps4 = epss[i]
                qlist = list(qs) if qs is not None else list(range(NQ))
                tiles = {}
                for q in qlist:
                    tiles[q] = pse.tile([128, SBLK], F32, name="eps", tag="eps")
                for qpair in [qlist[i : i + 2] for i in range(0, len(qlist), 2)]:
                    # stationary (j, q) constant across both halves: walrus can
                    # reuse the loaded weights for the second matmul
                    for j in range(KH // 2):
                        for q in qpair:
                            for half in range(SBLK // HB):
                                hsl = slice(half * HB, (half + 1) * HB)
                                nc.tensor.matmul(
                                    tiles[q][:, hsl],
                                    we_sb[:, j, q, :],
                                    et[:, 2 * j : 2 * j + 2, hsl],
                                    start=(j == 0),
                                    stop=(j == KH // 2 - 1),
                                    perf_mode=mybir.MatmulPerfMode.DoubleRowSwInterleave,
                                )
                for q in qlist:
                    eps4.append(tiles[q])
                if qs is None or qs[-1] == NQ - 1:
                    ets.pop(i)

            ens = {}

            def emit_tanh(i):
                b, sblk = blocks[i]
                en4 = []
                for q in range(NQ):
                    eps = epss[i][q]
                    en = enp.tile([128, SBLK], BF16)
                    nc.scalar.activation(
                        en[:],
                        eps[:],
                        TANH,
                        bias=hptb[:, q, b : b + 1],
                        scale=1.0 / WE_SCALE,
                    )
                    en4.append(en)
                ens[i] = en4
                del epss[i]

            def emit_v(i):
                # chunk c -> row c of ps_small[0:16] (bank 0).  Stationary is
                # [128,16] with v in column c and zeros elsewhere, so rows
                # other than c accumulate zero; chunk 0 q0 start=True clears
                # the bank once, everything after accumulates via has_written.
                NR = NCH * BC
                for half in range(SBLK // HB):
                    c = i * (SBLK // HB) + half
                    att_ps = ps_small[0:NR, 0:HB]
                    for q in range(NQ):
                        nc.tensor.matmul(
                            att_ps,
                            vm_sb[:, q, c, :],
                            ens[i][q][:, half * HB : (half + 1) * HB],
                            start=(c == 0 and q == 0),
                            stop=(q == NQ - 1),
                            skip_group_check=True,
                        )
                del ens[i]

            # prologue: sync queue carries only enc tiles (fp8, 256 KB each);
            # h_part matmuls interleave with block 0's e-matmuls so the tanh
            # bias is ready as early as possible
            load_block(0)
            hid_sb = constp.tile([128, KD, BC], BF16)
            nc.sync.dma_start(hid_sb[:], hid_d[:])
            nc.sync.dma_start(wh_sb[:, KD // 2 :, :], wh_d[:, KD // 2 :, :])
            load_block(1)
            hp_ps = ps_small[0:BC, 0:H]

            def emit_hp(ks):
                for k in ks:
                    nc.tensor.matmul(
                        hp_ps,
                        hid_sb[:, k, :],
                        wh_sb[:, k, :],
                        start=(k == 0),
                        stop=(k == KD - 1),
                    )

            emit_hp(range(KD))
            hp_sb = smallp.tile([BC, H], F32)
            nc.vector.tensor_copy(hp_sb[:], hp_ps)

            # transpose to [128, q, b] via PE, fold in b_attn -> tanh bias
            for q in range(NQ):
                hpt_ps = ps_small[:, q * BC : (q + 1) * BC]
                nc.tensor.transpose(
                    hpt_ps, hp_sb[:, q * 128 : (q + 1) * 128], id_sb[:]
                )
                nc.vector.tensor_scalar_add(
                    hptb[:, q, :], hpt_ps, ba_sb[:, q : q + 1]
                )
            emit_emm(0)

            # steady state, one-block skew: e-matmuls of block i+1 sit ahead of
            # block i's tanh-dependent v-dots in the PE stream
            for i in range(NBLOCKS - 1):
                if i + 2 < NBLOCKS:
                    load_block(i + 2)
                if i + 1 < NBLOCKS:
                    emit_emm(i + 1)
                emit_tanh(i)
                if i >= 1:
                    emit_v(i - 1)
            emit_v(NBLOCKS - 2)
            # final block: tanh of quadrant q immediately feeds its 2 v-dot
            # matmuls, so the drain is one tanh + pipelining instead of four
            last = NBLOCKS - 1
            NR = NCH * BC
            lb, _ = blocks[last]
            for q in range(NQ):
                en = enp.tile([128, SBLK], BF16, name="en_tail")
                nc.scalar.activation(
                    en[:],
                    epss[last][q][:],
                    TANH,
                    bias=hptb[:, q, lb : lb + 1],
                    scale=1.0 / WE_SCALE,
                )
                for half in range(SBLK // HB):
                    c = last * (SBLK // HB) + half
                    nc.tensor.matmul(
                        ps_small[0:NR, 0:HB],
                        vm_sb[:, q, c, :],
                        en[:, half * HB : (half + 1) * HB],
                        start=False,
                        stop=(q == NQ - 1),
                        skip_group_check=True,
                    )
            del epss[last]

            # one-shot softmax over all 16 logit rows [16, 512] in psum
            nc.scalar.activation(
                ex16[:], ps_small[0:NR, 0:HB], EXP, accum_out=csum[:]
            )
            rs_ps = pse.tile([128, SBLK], F32, name="eps", tag="eps")
            nc.tensor.matmul(
                rs_ps[0:NR, 0:1], bd_sb[:], csum[:], start=True, stop=True
            )
            nc.vector.reciprocal(rs16[:], rs_ps[0:NR, 0:1])
            nc.vector.tensor_scalar_mul(out16[:], ex16[:], rs16[:])
            for b in range(BC):
                nc.sync.dma_start(out_d[b], out16[NCH * b : NCH * b + NCH, :])

    nc.compile()
    return nc


def _get_nc():
    global _NC_CACHE
    if _NC_CACHE is None:
        _NC_CACHE = _build()
    return _NC_CACHE


def _prep_inputs(hidden, encoder_outputs, W_attn, b_attn, v):
    f = np.float32
    W_h = np.asarray(W_attn[:DH], dtype=f)
    W_e = np.asarray(W_attn[DH:], dtype=f)
    import ml_dtypes
    bf = ml_dtypes.bfloat16
    f8 = ml_dtypes.float8_e4m3
    wh_prep = np.ascontiguousarray(W_h.reshape(KD, 128, H).transpose(1, 0, 2)).astype(bf)
    we_raw = np.clip(
        np.ascontiguousarray(W_e.reshape(KH, 128, H).transpose(1, 0, 2)) * 64.0,
        -240.0, 240.0,
    ).astype(f)
    # DoubleRowSwInterleave layout: per partition [A127,B127,...,A0,B0]
    we_swi = np.zeros((128, KH // 2, NQ, 256), dtype=f)
    for j in range(KH // 2):
        for q in range(NQ):
            A = we_raw[:, 2 * j, q * 128 : (q + 1) * 128]
            Bm = we_raw[:, 2 * j + 1, q * 128 : (q + 1) * 128]
            we_swi[:, j, q, 0::2] = A[:, ::-1]
            we_swi[:, j, q, 1::2] = Bm[:, ::-1]
    we_prep = we_swi.astype(f8)
    ba_prep = np.ascontiguousarray(np.asarray(b_attn, dtype=f).reshape(NQ, 128).T)
    v_prep = np.ascontiguousarray(np.asarray(v, dtype=f).reshape(NQ, 128).T).astype(bf)
    NR = NCH * BC
    vmat = np.zeros((128, NQ, NR, NR), dtype=f)
    for c in range(NR):
        vmat[:, :, c, c] = np.asarray(v, dtype=f).reshape(NQ, 128).T
    vmat_prep = vmat.astype(bf)
    ident = np.eye(BC, dtype=f)
    NR = NCH * BC
    bd = np.zeros((NR, NR), dtype=f)
    for g in range(BC):
        bd[NCH * g : NCH * g + NCH, NCH * g : NCH * g + NCH] = 1.0
    hidden = np.asarray(hidden, dtype=f)
    encoder_outputs = np.asarray(encoder_outputs, dtype=f)

    in_maps = []
    for c in range(NCORES):
        b0 = c * BC
        hc = hidden[b0 : b0 + BC]                       # [BC, DH]
        hid_prep = np.ascontiguousarray(
            hc.T.reshape(KD, 128, BC).transpose(1, 0, 2)
        ).astype(bf)
        ec = encoder_outputs[:, b0 : b0 + BC, :]        # [S, BC, H]
        # enc_prep[b, sblk, p, k, si] = ec[sblk*SBLK+si, b, k*128+p]
        enc_prep = np.clip(
            np.ascontiguousarray(
                ec.transpose(1, 0, 2)
                .reshape(BC, NBLK, SBLK, KH, 128)
                .transpose(0, 1, 4, 3, 2)
            ),
            -240.0, 240.0,
        ).astype(ml_dtypes.float8_e4m3)
        in_maps.append(
            {
                "enc_t": enc_prep,
                "hid_t": hid_prep,
                "w_h": wh_prep,
                "w_e": we_prep,
                "b_attn": ba_prep,
                "v": v_prep,
                "ident": ident,
                "bdiag": bd,
                "vmat": vmat_prep,
            }
        )
    return in_maps


def _run(inputs, trace=False, **kw):
    nc = _get_nc()
    in_maps = _prep_inputs(
        inputs["hidden"],
        inputs["encoder_outputs"],
        inputs["W_attn"],
        inputs["b_attn"],
        inputs["v"],
    )
    res = run_bass_kernel_spmd(
        nc, in_maps, core_ids=list(range(NCORES)), trace=trace, **kw
    )
    out = np.concatenate(
        [r["out"].reshape(BC, S) for r in res.results], axis=0
    ).astype(np.float32)
    return out, res


def kernel(**inputs):
    out, _ = _run(inputs, trace=False)
    return out

